# revision 6
# baseline (speedup 1.0000x reference)
"""Cross-attention kernel for Trainium2 (8 NeuronCores, SPMD).

Problem: B=4, Nq=1024, Nk=2048, D=512, 8 heads x 64 head-dim, fp32,
full-tensor bias added to scores before softmax.

Sharding: (batch, query-half) -> 8 disjoint shards, one per core. Each core
computes its own (512, 512) slice of the output; no collectives needed.
K/V projections are computed redundantly on the two cores sharing a batch.

Device layout: attention tensors kept transposed (feature/key dim on
partitions) so every matmul contraction lands on the partition axis:
  QT[d, q] = (SCALE*Wq) @ xT          KT[d, k] = Wk @ ctxT
  V[k, i]  = ctxT.T @ Wv.T
  ST[k, q] = KT_h.T @ QT_h            (the two heads of a pair sit in PE row
                                       groups 0-63/64-127 and run concurrently)
  E = exp(ST) * exp(biasT)            (ACT exp; DVE multiply with a stride-0
                                       broadcast read of the bias tile; no
                                       shift needed: logits <= ~10 so exp
                                       fits fp16)
  ...or for designated chunks, a Schraudolph-style punned exp on DVE:
  E = bitcast_f16(int16((ST*c1) + b'')) with b'' = bias*c1 + c2 (host)
  out2T[i(+1), q] = [V_h | 1].T @ E   (ones column yields softmax row-sums
                                       in the same accumulation)
  OT = o2[0:64] * pbcast(recip(sum))  (reciprocal on DVE straight from PSUM,
                                       broadcast across partitions on GPSIMD)
  yT[d, q] = Wo @ OT + bo             (fp16 writeback; host converts)
PSUM evacuations (K/V/Q) ride gpsimd-initiated casting DMAs instead of DVE.
Matmul operands are fp16 (fp32 PSUM accumulate).
"""

import numpy as np
import concourse.bass as bass
import concourse.bacc as bacc
import concourse.mybir as mybir
import concourse.tile as tile
from concourse import bass_utils

HEADS = 8
DH = 64
D = 512
NQ = 512          # queries per core (Nq=1024 split in halves)
NK = 2048
KC = NK // 128    # 16 key chunks
SCALE = DH ** -0.5

F32 = mybir.dt.float32
F16 = mybir.dt.float16
I16 = mybir.dt.int16
AF = mybir.ActivationFunctionType
ALU = mybir.AluOpType

# ---- tuning switches ----
EVAC_DMA = False          # K/V PSUM->SBUF evacuation via gpsimd casting DMA
WARMUP = True            # PE p-state warmup matmuls during initial loads
# chunks (per head-pair) that use the punned-exp DVE path instead of ACT exp
SCH_CHUNKS = frozenset()
# Schraudolph fp16 punning constants: exp(x) ~= bitcast(round(x*C1 + C2))
C1 = 1024.0 * 1.4426950408889634
C2 = 15.0 * 1024.0 - 0.043 * C1      # center the 2^frac vs 1+frac lobe


def _bcast2(ap, n):
    """[128, F] -> [128, n, F] with a step-0 middle dim."""
    return bass.AP(ap.tensor, ap.offset, [ap.ap[0], [0, n], ap.ap[1]])


def _build_nc():
    nc = bacc.Bacc("TRN2", target_bir_lowering=False, debug=False)

    xT_d = nc.dram_tensor("xT", [D, NQ], F16, kind="ExternalInput")
    ctxT_d = nc.dram_tensor("ctxT", [D, NK], F16, kind="ExternalInput")
    # per-chunk rows: exp(bias).T for ACT chunks, bias.T*C1+C2 for SCH chunks
    bmix_d = nc.dram_tensor("bmix", [NK, NQ], F16, kind="ExternalInput")
    wqT_d = nc.dram_tensor("wqT", [D, D], F16, kind="ExternalInput")
    wkT_d = nc.dram_tensor("wkT", [D, D], F16, kind="ExternalInput")
    wvT_d = nc.dram_tensor("wvT", [D, D], F16, kind="ExternalInput")
    woT_d = nc.dram_tensor("woT", [D, D], F16, kind="ExternalInput")
    bo_d = nc.dram_tensor("bo", [D, 1], F32, kind="ExternalInput")
    yT_d = nc.dram_tensor("yT", [D, NQ], F16, kind="ExternalOutput")

    with tile.TileContext(nc) as tc, nc.allow_low_precision(
            reason="fp16 matmul operands, fp32 accumulation"):
        with (
            tc.tile_pool(name="const", bufs=1) as const,
            tc.tile_pool(name="main", bufs=1) as main,
            tc.tile_pool(name="work", bufs=6) as work,
            tc.tile_pool(name="norm", bufs=3) as norm,
            tc.tile_pool(name="ctxp", bufs=1) as ctxp,
        ):
            # ---- loads: spread across queues; ctx in column slices so the
            # first K-projection group starts after ~1/4 of ctx ----
            wq = [const.tile([128, D], F16, name=f"wq{i}", tag=f"wq{i}") for i in range(4)]
            wk = [const.tile([128, D], F16, name=f"wk{i}", tag=f"wk{i}") for i in range(4)]
            wv = [const.tile([128, D], F16, name=f"wv{i}", tag=f"wv{i}") for i in range(4)]
            wo = [const.tile([128, D], F16, name=f"wo{i}", tag=f"wo{i}") for i in range(4)]
            bo_sb = [const.tile([128, 1], F32, name=f"bo{i}", tag=f"bo{i}") for i in range(4)]
            onesF = const.tile([128, 1], F32, name="onesF", tag="onesF")
            nc.vector.memset(onesF, 1.0)
            warm = const.tile([128, D], F16, name="warm", tag="warm")
            nc.vector.memset(warm, 0.25)
            ctx = [ctxp.tile([128, NK], F16, name=f"ctx{i}", tag=f"ctx{i}") for i in range(4)]
            xts = [ctxp.tile([128, NQ], F16, name=f"xts{i}", tag=f"xts{i}") for i in range(4)]
            for i in range(4):
                sl = slice(i * 128, (i + 1) * 128)
                nc.sync.dma_start(out=wk[i], in_=wkT_d[sl, :])
            for nt in range(4):
                nsl = slice(nt * 512, (nt + 1) * 512)
                for i in range(4):
                    sl = slice(i * 128, (i + 1) * 128)
                    nc.sync.dma_start(out=ctx[i][:, nsl], in_=ctxT_d[sl, nsl])
            for i in range(4):
                sl = slice(i * 128, (i + 1) * 128)
                nc.scalar.dma_start(out=xts[i], in_=xT_d[sl, :])
                nc.scalar.dma_start(out=wq[i], in_=wqT_d[sl, :])
                nc.scalar.dma_start(out=wv[i], in_=wvT_d[sl, :])

            KT = [main.tile([128, NK], F16, name=f"KT{i}", tag=f"KT{i}") for i in range(4)]
            QT = [main.tile([128, NQ], F16, name=f"QT{i}", tag=f"QT{i}") for i in range(4)]
            OT = [main.tile([128, NQ], F16, name=f"OT{i}", tag=f"OT{i}") for i in range(4)]
            Vo = [main.tile([128, HEADS, DH + 1], F16, name=f"Vo{c}", tag=f"Vo{c}")
                  for c in range(KC)]
            eB = [main.tile([128, NQ], F16, name=f"eB{c}", tag=f"eB{c}") for c in range(KC)]
            for c in range(KC):
                nc.vector.tensor_copy(
                    Vo[c][:, :, DH], onesF[:, 0:1].broadcast_to([128, HEADS]))
            for c in range(6):
                nc.sync.dma_start(out=eB[c], in_=bmix_d[c * 128:(c + 1) * 128, :])
            for i in range(4):
                sl = slice(i * 128, (i + 1) * 128)
                nc.scalar.dma_start(out=wo[i], in_=woT_d[sl, :])
                nc.scalar.dma_start(out=bo_sb[i], in_=bo_d[sl, :])

            def k_proj_group(psA, mi, nt):
                msl = slice(mi * 128, (mi + 1) * 128)
                nsl = slice(nt * 512, (nt + 1) * 512)
                ps = psA.tile([128, 512], F32, name="proj", tag="proj")
                for ki in range(4):
                    nc.tensor.matmul(
                        ps, wk[ki][:, msl], ctx[ki][:, nsl],
                        start=(ki == 0), stop=(ki == 3))
                if EVAC_DMA:
                    nc.gpsimd.tensor_copy(KT[mi][:, nsl], ps)
                else:
                    nc.vector.tensor_copy(KT[mi][:, nsl], ps)

            def v_proj_group(psA, c):
                csl = slice(c * 128, (c + 1) * 128)
                ps = psA.tile([128, 512], F32, name="vproj", tag="proj")
                for ki in range(4):
                    nc.tensor.matmul(
                        ps, ctx[ki][:, csl], wv[ki],
                        start=(ki == 0), stop=(ki == 3))
                if EVAC_DMA:
                    nc.gpsimd.tensor_copy(
                        Vo[c][:, :, 0:DH],
                        ps.rearrange("p (h d) -> p h d", h=HEADS))
                else:
                    nc.vector.tensor_copy(
                        Vo[c][:, :, 0:DH],
                        ps.rearrange("p (h d) -> p h d", h=HEADS))

            def q_proj_group(psA, mi):
                msl = slice(mi * 128, (mi + 1) * 128)
                ps = psA.tile([128, 512], F32, name="proj", tag="proj")
                for ki in range(4):
                    nc.tensor.matmul(
                        ps, wq[ki][:, msl], xts[ki],
                        start=(ki == 0), stop=(ki == 3))
                nc.vector.tensor_copy(QT[mi], ps)

            # ---- PE p-state warmup while DMAs stream ----
            if WARMUP:
                with tc.tile_pool(name="psW", bufs=1, space="PSUM") as psW:
                    pw = psW.tile([128, 512], F32, name="pw", tag="pw")
                    for r in range(14):
                        nc.tensor.matmul(pw, warm[:, 0:128], warm,
                                         start=True, stop=True)

            # ---- upfront projections: K/Q for pair 0, first two V ----
            with tc.tile_pool(name="psA0", bufs=3, space="PSUM") as psA0:
                for nt in range(4):
                    k_proj_group(psA0, 0, nt)
                q_proj_group(psA0, 0)
                for c in range(2):
                    v_proj_group(psA0, c)

            # ---- attention (head pairs) with interleaved projections ----
            with (
                tc.tile_pool(name="psS", bufs=2, space="PSUM") as psS,
                tc.tile_pool(name="psO", bufs=3, space="PSUM") as psO,
                tc.tile_pool(name="psA", bufs=1, space="PSUM") as psA,
            ):
                for hp in range(4):
                    h0, h1 = 2 * hp, 2 * hp + 1
                    lo, hi = slice(0, DH), slice(DH, 128)
                    o2a = psO.tile([DH + 1, NQ], F32, name="o2a", tag="o2")
                    o2b = psO.tile([DH + 1, NQ], F32, name="o2b", tag="o2")
                    for c in range(KC):
                        csl = slice(c * 128, (c + 1) * 128)
                        s = psS.tile([128, 2, NQ], F32, name="s", tag="s")
                        nc.tensor.matmul(
                            s[:, 0, :], KT[hp][lo, csl], QT[hp][lo, :],
                            start=True, stop=True)
                        nc.tensor.matmul(
                            s[:, 1, :], KT[hp][hi, csl], QT[hp][hi, :],
                            start=True, stop=True)
                        et = work.tile([128, 2, NQ], F16, name="et", tag="et")
                        if c in SCH_CHUNKS:
                            # punned exp: et = bitcast_f16(i16(s*C1 + b''))
                            nc.vector.scalar_tensor_tensor(
                                et.bitcast(I16), s, float(C1),
                                _bcast2(eB[c][:], 2),
                                ALU.mult, ALU.add)
                        else:
                            e1 = work.tile([128, 2, NQ], F16, name="e1", tag="e1")
                            nc.scalar.activation(e1, s, AF.Exp)
                            nc.vector.tensor_mul(et, e1, _bcast2(eB[c][:], 2))
                        nc.tensor.matmul(
                            o2a, Vo[c][:, h0, :], et[:, 0, :],
                            start=(c == 0), stop=(c == KC - 1))
                        nc.tensor.matmul(
                            o2b, Vo[c][:, h1, :], et[:, 1, :],
                            start=(c == 0), stop=(c == KC - 1))
                        if hp == 0 and c < 10:
                            nc.sync.dma_start(
                                out=eB[c + 6],
                                in_=bmix_d[(c + 6) * 128:(c + 7) * 128, :])
                        # TensorE filler: remaining V groups ride inside
                        # pair 0; each pair also preloads the next pair's K/Q
                        if hp == 0:
                            if c <= 13:
                                v_proj_group(psA, c + 2)
                            if c in (3, 6, 9, 12):
                                k_proj_group(psA, 1, c // 3 - 1)
                            elif c == 14:
                                q_proj_group(psA, 1)
                        elif hp < 3:
                            if c in (1, 3, 5, 7):
                                k_proj_group(psA, hp + 1, (c - 1) // 2)
                            elif c == 9:
                                q_proj_group(psA, hp + 1)
                    # normalize each head of the pair: reciprocal of the
                    # ones-row straight from PSUM, partition-broadcast on
                    # GPSIMD, multiply rows 0-63 from PSUM on DVE
                    for h, o2 in ((h0, o2a), (h1, o2b)):
                        rsl = slice((h % 2) * DH, (h % 2) * DH + DH)
                        sr = norm.tile([1, NQ], F16, name="sr", tag="sr")
                        nc.vector.reciprocal(sr, o2[DH:DH + 1, :])
                        cbs = norm.tile([DH, NQ], F16, name="cbs", tag="cbs")
                        nc.gpsimd.partition_broadcast(cbs[:], sr[:])
                        nc.vector.tensor_mul(OT[hp][rsl, :], o2[0:DH, :], cbs)

            # ---- output projection + bias (ki-outer: the ki<3 partial
            # sums run while the last head pair is still normalizing) ----
            with tc.tile_pool(name="psY", bufs=1, space="PSUM") as psY:
                pss = [psY.tile([128, NQ], F32, name=f"yTp{mi}", tag=f"yTp{mi}")
                       for mi in range(4)]
                for ki in range(4):
                    for mi in range(4):
                        msl = slice(mi * 128, (mi + 1) * 128)
                        nc.tensor.matmul(
                            pss[mi], wo[ki][:, msl], OT[ki],
                            start=(ki == 0), stop=(ki == 3))
                for mi in range(4):
                    msl = slice(mi * 128, (mi + 1) * 128)
                    ysb = work.tile([128, NQ], F16, name="ysb", tag="ysb")
                    nc.vector.tensor_scalar_add(ysb, pss[mi], bo_sb[mi])
                    nc.sync.dma_start(out=yT_d[msl, :], in_=ysb)

    nc.compile()
    return nc


_NC_CACHE = {}


def _get_nc():
    if "nc" not in _NC_CACHE:
        _NC_CACHE["nc"] = _build_nc()
    return _NC_CACHE["nc"]


def make_in_maps(x, context, bias, Wq, Wk, Wv, Wo, bo):
    x = np.asarray(x, dtype=np.float32)
    context = np.asarray(context, dtype=np.float32)
    bias = np.asarray(bias, dtype=np.float32)
    wqT = np.ascontiguousarray((np.asarray(Wq) * SCALE).T).astype(np.float16)
    wkT = np.ascontiguousarray(np.asarray(Wk).T).astype(np.float16)
    wvT = np.ascontiguousarray(np.asarray(Wv).T).astype(np.float16)
    woT = np.ascontiguousarray(np.asarray(Wo).T).astype(np.float16)
    bo2 = np.ascontiguousarray(np.asarray(bo, dtype=np.float32).reshape(D, 1))

    sch_rows = np.zeros(NK, dtype=bool)
    for c in SCH_CHUNKS:
        sch_rows[c * 128:(c + 1) * 128] = True

    in_maps = []
    for core in range(8):
        b, half = core // 2, core % 2
        qs = half * NQ
        bT = bias[b, qs:qs + NQ, :].T  # [NK, NQ]
        bmix = np.where(sch_rows[:, None], bT * C1 + C2, np.exp(bT))
        in_maps.append({
            "xT": np.ascontiguousarray(x[b, qs:qs + NQ, :].T).astype(np.float16),
            "ctxT": np.ascontiguousarray(context[b].T).astype(np.float16),
            "bmix": np.ascontiguousarray(bmix).astype(np.float16),
            "wqT": wqT, "wkT": wkT, "wvT": wvT, "woT": woT, "bo": bo2,
        })
    return in_maps


def kernel(x, context, bias, Wq, Wk, Wv, Wo, bo):
    nc = _get_nc()
    in_maps = make_in_maps(x, context, bias, Wq, Wk, Wv, Wo, bo)
    res = bass_utils.run_bass_kernel_spmd(
        nc, in_maps, core_ids=list(range(8)), trace=False)

    out = np.empty((4, 2 * NQ, D), dtype=np.float32)
    for core in range(8):
        b, half = core // 2, core % 2
        qs = half * NQ
        out[b, qs:qs + NQ, :] = res.results[core]["yT"].T.astype(np.float32)
    return out
